# revision 25
# baseline (speedup 1.0000x reference)
"""Trainium2 Bass kernel for causal multi-head attention (B=4, N=2048, DIM=1024, H=16, DH=64).

Sharding: 8 cores = (batch, head-group) pairs. Core c handles batch c//2 and
heads (c%2)*8 .. (c%2)*8+7.  Each core computes QKV projection for its 8 heads,
causal flash-attention, and a partial output projection (its heads' rows of
w_out).  The host sums the two partial outputs per batch and adds b_out.

Device-side layout choices (per core):
  - x is fed pre-transposed as xT [DIM, N] bf16 (host prep), so the QKV
    projection contraction (over DIM) sits on partitions with no on-device
    transpose.
  - Q^T, K^T computed as [head_dim, tok] (weights-stationary matmuls) so that
    scores can be computed directly as S^T = K^T.T @ Q^T with contraction dh=64.
  - S^T tiles are [128 k-tok, 512 q-tok]; softmax denominator comes free by
    augmenting V with a ones column: O^T_aug = [V | 1].T @ exp(S^T).
  - V computed as [tok, dh] (x-stationary matmuls), stored interleaved with the
    ones column: per k-tile [128, 8*65].
  - Causal masking: multiply exp(S^T) by precomputed 0/1 bf16 tiles on the
    diagonal blocks only (exp of a finite garbage score times 0 is exactly 0).
  - Key-padding mask folds into V_aug: V_aug row k scaled by mask[k] zeroes both
    numerator and denominator contributions of masked keys.
"""

import numpy as np
import ml_dtypes

B, N, DIM, H, DH = 4, 2048, 1024, 16, 64
HPC = 8            # heads per core
HD = HPC * DH      # 512 head dims per core
NCORES = 8
BF16 = ml_dtypes.bfloat16

TOK_TILE = 128     # k-token tile (partition dim of S^T)
QCHUNK = 512       # q-token chunk (free dim of S^T)
NKT = N // TOK_TILE       # 16 k tiles
NQC = N // QCHUNK         # 4 q chunks
NQT = N // 128            # 16 q tiles (out-projection)
DCH = DIM // 128          # 8 contraction chunks over DIM
VROW = HPC * (DH + 1)     # 520: V_aug row elems per k-tile

_CACHE = {}


def _build_program():
    from contextlib import ExitStack
    import concourse.bass as bass
    import concourse.tile as tile
    from concourse import bacc, mybir

    dt = mybir.dt
    f32 = dt.float32
    bf16 = dt.bfloat16
    Exp = mybir.ActivationFunctionType.Exp

    nc = bacc.Bacc("TRN2", target_bir_lowering=False, debug=False,
                   enable_asserts=False, num_devices=NCORES)

    xT = nc.dram_tensor("xT", [DIM, N], bf16, kind="ExternalInput").ap()
    wq = nc.dram_tensor("wq", [DIM, HD], bf16, kind="ExternalInput").ap()
    wk = nc.dram_tensor("wk", [DIM, HD], bf16, kind="ExternalInput").ap()
    wv = nc.dram_tensor("wv", [DIM, HD], bf16, kind="ExternalInput").ap()
    wo = nc.dram_tensor("wo", [HD, DIM], bf16, kind="ExternalInput").ap()
    kpm = nc.dram_tensor("kpm", [N, 1], f32, kind="ExternalInput").ap()
    cmask_d = nc.dram_tensor("cmask", [4 * 128, QCHUNK], bf16,
                             kind="ExternalInput").ap()
    out_d = nc.dram_tensor("out", [N, DIM], f32, kind="ExternalOutput").ap()

    with tile.TileContext(nc) as tc, ExitStack() as ctx:
        const = ctx.enter_context(tc.tile_pool(name="const", bufs=1))
        p_sbp = ctx.enter_context(tc.tile_pool(name="p_sbp", bufs=6))
        miscp = ctx.enter_context(tc.tile_pool(name="miscp", bufs=3))
        outp = ctx.enter_context(tc.tile_pool(name="outp", bufs=3))
        mm_ps = ctx.enter_context(tc.tile_pool(name="mm_ps", bufs=2, space="PSUM"))
        s_ps = ctx.enter_context(tc.tile_pool(name="s_ps", bufs=2, space="PSUM"))
        o_ps = ctx.enter_context(tc.tile_pool(name="o_ps", bufs=2, space="PSUM"))

        # ---- persistent SBUF tensors ----
        xT_sb = [const.tile([128, N], bf16, name=f"xTsb{c}") for c in range(DCH)]
        wq_sb = [const.tile([128, HD], bf16, name=f"wqsb{c}") for c in range(DCH)]
        wk_sb = [const.tile([128, HD], bf16, name=f"wksb{c}") for c in range(DCH)]
        wv_sb = [const.tile([128, HD], bf16, name=f"wvsb{c}") for c in range(DCH)]
        wo_sb = [const.tile([128, DIM], bf16, name=f"wosb{c}") for c in range(4)]
        # Q^T / K^T packed: chunk c holds heads 2c (parts 0-63) and 2c+1 (64-127)
        QT = [const.tile([128, N], bf16, name=f"QTsb{c}") for c in range(4)]
        KT = [const.tile([128, N], bf16, name=f"KTsb{c}") for c in range(4)]
        # V_aug: per k-tile block of 8*(64+1) cols
        V = const.tile([128, NKT * VROW], bf16, name="Vsb")
        # O^T packed like QT/KT
        OT = [const.tile([128, N], bf16, name=f"OTsb{c}") for c in range(4)]
        cmask = const.tile([128, 4 * QCHUNK], bf16, name="cmasksb")
        # key-padding mask: col t = mask[t*128 + p] (one tiny DMA, loaded
        # first so V-proj evacuations never wait behind the big weight loads)
        kpm_sb = const.tile([128, NKT], f32, name="kpmsb")

        sync = nc.sync
        sync.dma_start(
            kpm_sb.rearrange("p (t one) -> p t one", one=1),
            kpm.rearrange("(t p) one -> p t one", p=128),
        )

        # ---- load inputs (wv+xT first so V projection can start ASAP;
        # full-chunk DMAs only — small column-split loads are inefficient) ----
        for c in range(DCH):
            sync.dma_start(wv_sb[c][:], wv[c * 128:(c + 1) * 128, :])
            sync.dma_start(xT_sb[c][:], xT[c * 128:(c + 1) * 128, :])
        for c in range(DCH):
            sync.dma_start(wq_sb[c][:], wq[c * 128:(c + 1) * 128, :])
            sync.dma_start(wk_sb[c][:], wk[c * 128:(c + 1) * 128, :])
        for c in range(4):
            sync.dma_start(wo_sb[c][:], wo[c * 128:(c + 1) * 128, :])
        # cmask DRAM row r*128+k, col q  ->  SBUF part k, col r*512+q
        sync.dma_start(
            cmask.rearrange("p (r q) -> p r q", r=4),
            cmask_d.rearrange("(r p) q -> p r q", p=128),
        )

        # ---- V projection: V[tok, dh] via x-stationary matmuls ----
        def v_proj():
            for kt in range(NKT):
                kpm_t = kpm_sb[:, kt:kt + 1]
                ps = mm_ps.tile([128, 512], f32, tag="mm", name="ps")
                for c in range(DCH):
                    nc.tensor.matmul(
                        ps[:], xT_sb[c][:, kt * 128:(kt + 1) * 128],
                        wv_sb[c][:],
                        start=(c == 0), stop=(c == DCH - 1))
                vblk = V[:, kt * VROW:(kt + 1) * VROW].rearrange(
                    "p (h c) -> p h c", c=DH + 1)
                # data cols, scaled by key-padding mask
                nc.vector.tensor_scalar_mul(
                    vblk[:, :, 0:DH],
                    ps.rearrange("p (h c) -> p h c", c=DH),
                    kpm_t[:, 0:1])
                # ones column = mask value (free-dim stride-0 broadcast read)
                nc.vector.tensor_copy(vblk[:, :, DH:DH + 1].squeeze(),
                                      kpm_t[:, 0:1].broadcast_to([128, HPC]))

        def qk_proj(c):
            for tcx in range(NQC):
                tsl = slice(tcx * QCHUNK, (tcx + 1) * QCHUNK)
                psq = mm_ps.tile([128, 512], f32, tag="mm", name="psq")
                for d in range(DCH):
                    nc.tensor.matmul(
                        psq[:], wq_sb[d][:, c * 128:(c + 1) * 128],
                        xT_sb[d][:, tsl],
                        start=(d == 0), stop=(d == DCH - 1))
                nc.vector.tensor_copy(QT[c][:, tsl], psq[:])
                psk = mm_ps.tile([128, 512], f32, tag="mm", name="psk")
                for d in range(DCH):
                    nc.tensor.matmul(
                        psk[:], wk_sb[d][:, c * 128:(c + 1) * 128],
                        xT_sb[d][:, tsl],
                        start=(d == 0), stop=(d == DCH - 1))
                nc.vector.tensor_copy(KT[c][:, tsl], psk[:])

        def attend(h, qc):
            c = h // 2
            po = (h % 2) * 64          # partition offset within chunk
            qt_h = QT[c][po:po + 64, :]
            kt_h = KT[c][po:po + 64, :]
            qsl = slice(qc * QCHUNK, (qc + 1) * QCHUNK)
            pso = o_ps.tile([DH + 1, 512], f32, tag="o", name="pso")
            nkt = 4 * qc + 4
            for kp in range(nkt // 2):
                ps2 = s_ps.tile([128, 1024], f32, tag="s", name="ps2")
                for j in (0, 1):
                    kt = 2 * kp + j
                    nc.tensor.matmul(
                        ps2[:, j * 512:(j + 1) * 512],
                        kt_h[:, kt * 128:(kt + 1) * 128],
                        qt_h[:, qsl],
                        start=True, stop=True)
                p2 = p_sbp.tile([128, 1024], bf16, tag="p", name="p2")
                nc.scalar.activation(p2[:], ps2[:], Exp)
                r = 2 * kp - 4 * qc
                if r >= 0:  # diagonal pair: apply causal 0/1 mask
                    nc.vector.tensor_mul(
                        p2[:], p2[:],
                        cmask[:, r * QCHUNK:(r + 2) * QCHUNK])
                for j in (0, 1):
                    kt = 2 * kp + j
                    nc.tensor.matmul(
                        pso[:],
                        V[:, kt * VROW + h * (DH + 1):
                           kt * VROW + (h + 1) * (DH + 1)],
                        p2[:, j * 512:(j + 1) * 512],
                        start=(kt == 0), stop=(kt == nkt - 1),
                        skip_group_check=True)
            # normalize: O^T[0:64] * (1 / rowsum row 64)
            # (stage rowsum into SBUF: custom-DVE recip needs SBUF in)
            rsum = miscp.tile([1, 512], f32, tag="rsum", name="rsum")
            nc.vector.tensor_copy(rsum[:], pso[DH:DH + 1, :])
            recip = miscp.tile([1, 512], f32, tag="recip", name="recip")
            nc.vector.reciprocal_approx_fast(recip[:], rsum[:])
            bcast = miscp.tile([64, 512], f32, tag="bcast", name="bcast")
            # replicate the recip row to 64 partitions via SBUF->SBUF DMA
            # (gpsimd partition_broadcast costs ~1.9us incl drain; DMA is
            # off-engine and overlappable)
            sync.dma_start(bcast[:],
                           recip[0:1, :].unsqueeze(1).broadcast_to(
                               [1, 64, 512]))
            if po == 0:
                nc.vector.tensor_mul(OT[c][0:64, qsl],
                                     pso[0:DH, :], bcast[:])
            else:
                otmp = miscp.tile([64, 512], bf16, tag="otmp", bufs=3,
                                  name="otmp")
                nc.vector.tensor_mul(otmp[:], pso[0:DH, :], bcast[:])
                # partition shift 0->64 needs a DMA, engines can't shift
                sync.dma_start(OT[c][64:128, qsl], otmp[:])

        def out_proj(qt):
            y_sb = outp.tile([128, DIM], f32, tag="y", name="y_sb")
            for oc in range(2):
                psy = mm_ps.tile([128, 512], f32, tag="mm", name="psy")
                for cc in range(4):
                    nc.tensor.matmul(
                        psy[:], OT[cc][:, qt * 128:(qt + 1) * 128],
                        wo_sb[cc][:, oc * 512:(oc + 1) * 512],
                        start=(cc == 0), stop=(cc == 3))
                nc.vector.tensor_copy(y_sb[:, oc * 512:(oc + 1) * 512],
                                      psy[:])
            sync.dma_start(out_d[qt * 128:(qt + 1) * 128, :], y_sb[:])

        # ---- proj chunk c then its two heads (overlaps ACT exp with PE
        # proj of later chunks); last pair goes qc-major with inline Y ----
        v_proj()
        qk_proj(0)
        for h in (0, 1):
            for qc in range(NQC):
                attend(h, qc)
        for c in (1, 2):
            qk_proj(c)
            for h in (2 * c, 2 * c + 1):
                for qc in range(NQC):
                    attend(h, qc)
        qk_proj(3)
        for qc in range(NQC):
            attend(7, qc)
            attend(6, qc)
            for qt in range(4 * qc, 4 * qc + 4):
                out_proj(qt)

    nc.compile()
    return nc


def _get_program():
    if "nc" not in _CACHE:
        _CACHE["nc"] = _build_program()
    return _CACHE["nc"]


def _prep_inputs(x, mask, w_qkv, w_out):
    """Build the 8 per-core input maps (host-side sharding)."""
    scale = DH ** -0.5
    # causal keep-mask patterns for the 4 diagonal k-tiles of a 512 q-chunk
    k_idx = np.arange(128)[:, None]
    q_idx = np.arange(QCHUNK)[None, :]
    cm = np.concatenate(
        [(q_idx >= r * 128 + k_idx) for r in range(4)], axis=0
    ).astype(BF16)  # [512, 512]

    xT = [np.ascontiguousarray(x[b].T).astype(BF16) for b in range(B)]
    in_maps = []
    for core in range(NCORES):
        b, hg = core // 2, core % 2
        cs = slice(hg * HD, (hg + 1) * HD)
        wq_s = (w_qkv[:, 0 * DIM:1 * DIM][:, cs] * scale).astype(BF16)
        wk_s = w_qkv[:, 1 * DIM:2 * DIM][:, cs].astype(BF16)
        wv_s = w_qkv[:, 2 * DIM:3 * DIM][:, cs].astype(BF16)
        wo_s = np.ascontiguousarray(w_out[cs, :]).astype(BF16)
        kpm = mask[b].astype(np.float32).reshape(N, 1)
        in_maps.append({
            "xT": xT[b], "wq": wq_s, "wk": wk_s, "wv": wv_s, "wo": wo_s,
            "kpm": np.ascontiguousarray(kpm), "cmask": cm,
        })
    return in_maps


def kernel(x, mask, w_qkv, w_out, b_out, _trace=False):
    from concourse import bass_utils

    x = np.asarray(x, dtype=np.float32)
    mask = np.asarray(mask)
    w_qkv = np.asarray(w_qkv, dtype=np.float32)
    w_out = np.asarray(w_out, dtype=np.float32)
    b_out = np.asarray(b_out, dtype=np.float32)

    nc = _get_program()
    in_maps = _prep_inputs(x, mask, w_qkv, w_out)
    res = bass_utils.run_bass_kernel_spmd(
        nc, in_maps, core_ids=list(range(NCORES)), trace=_trace)

    out = np.empty((B, N, DIM), dtype=np.float32)
    for b in range(B):
        out[b] = res.results[2 * b]["out"] + res.results[2 * b + 1]["out"] + b_out
    if _trace:
        return out, res
    return out


# revision 29
# speedup vs baseline: 1.2580x; 1.2580x over previous
"""Trainium2 Bass kernel for causal multi-head attention (B=4, N=2048, DIM=1024, H=16, DH=64).

Sharding: 8 cores = (batch, head-group) pairs. Core c handles batch c//2 and
heads (c%2)*8 .. (c%2)*8+7.  Each core computes QKV projection for its 8 heads,
causal flash-attention, and a partial output projection (its heads' rows of
w_out).  The host sums the two partial outputs per batch and adds b_out.

Device-side layout choices (per core):
  - x is fed pre-transposed as xT [DIM, N] bf16 (host prep), so the QKV
    projection contraction (over DIM) sits on partitions with no on-device
    transpose.
  - Q^T, K^T computed as [head_dim, tok] (weights-stationary matmuls) so that
    scores can be computed directly as S^T = K^T.T @ Q^T with contraction dh=64.
  - S^T tiles are [128 k-tok, 512 q-tok]; softmax denominator comes free by
    augmenting V with a ones column: O^T_aug = [V | 1].T @ exp(S^T).
  - V computed as [tok, dh] (x-stationary matmuls), stored interleaved with the
    ones column: per k-tile [128, 8*65].
  - Causal masking: multiply exp(S^T) by precomputed 0/1 bf16 tiles on the
    diagonal blocks only (exp of a finite garbage score times 0 is exactly 0).
  - Key-padding mask folds into V_aug: V_aug row k scaled by mask[k] zeroes both
    numerator and denominator contributions of masked keys.
"""

import numpy as np
import ml_dtypes

B, N, DIM, H, DH = 4, 2048, 1024, 16, 64
HPC = 8            # heads per core
HD = HPC * DH      # 512 head dims per core
NCORES = 8
BF16 = ml_dtypes.bfloat16

TOK_TILE = 128     # k-token tile (partition dim of S^T)
QCHUNK = 512       # q-token chunk (free dim of S^T)
NKT = N // TOK_TILE       # 16 k tiles
NQC = N // QCHUNK         # 4 q chunks
NQT = N // 128            # 16 q tiles (out-projection)
DCH = DIM // 128          # 8 contraction chunks over DIM
VROW = HPC * (DH + 1)     # 520: V_aug row elems per k-tile

_CACHE = {}


def _build_program():
    from contextlib import ExitStack
    import concourse.bass as bass
    import concourse.tile as tile
    from concourse import bacc, mybir

    dt = mybir.dt
    f32 = dt.float32
    bf16 = dt.bfloat16
    Exp = mybir.ActivationFunctionType.Exp

    nc = bacc.Bacc("TRN2", target_bir_lowering=False, debug=False,
                   enable_asserts=False, num_devices=NCORES)

    xT = nc.dram_tensor("xT", [DIM, N], bf16, kind="ExternalInput").ap()
    wq = nc.dram_tensor("wq", [DIM, HD], bf16, kind="ExternalInput").ap()
    wk = nc.dram_tensor("wk", [DIM, HD], bf16, kind="ExternalInput").ap()
    wv = nc.dram_tensor("wv", [DIM, HD], bf16, kind="ExternalInput").ap()
    wo = nc.dram_tensor("wo", [HD, DIM], bf16, kind="ExternalInput").ap()
    kpm = nc.dram_tensor("kpm", [N, 1], f32, kind="ExternalInput").ap()
    cmask_d = nc.dram_tensor("cmask", [4 * 128, QCHUNK], bf16,
                             kind="ExternalInput").ap()
    out_d = nc.dram_tensor("out", [N, DIM], f32, kind="ExternalOutput").ap()

    with tile.TileContext(nc) as tc, ExitStack() as ctx:
        const = ctx.enter_context(tc.tile_pool(name="const", bufs=1))
        p_sbp = ctx.enter_context(tc.tile_pool(name="p_sbp", bufs=6))
        miscp = ctx.enter_context(tc.tile_pool(name="miscp", bufs=3))
        outp = ctx.enter_context(tc.tile_pool(name="outp", bufs=3))
        mm_ps = ctx.enter_context(tc.tile_pool(name="mm_ps", bufs=2, space="PSUM"))
        s_ps = ctx.enter_context(tc.tile_pool(name="s_ps", bufs=2, space="PSUM"))
        o_ps = ctx.enter_context(tc.tile_pool(name="o_ps", bufs=2, space="PSUM"))

        # ---- persistent SBUF tensors ----
        xT_sb = [const.tile([128, N], bf16, name=f"xTsb{c}") for c in range(DCH)]
        wq_sb = [const.tile([128, HD], bf16, name=f"wqsb{c}") for c in range(DCH)]
        wk_sb = [const.tile([128, HD], bf16, name=f"wksb{c}") for c in range(DCH)]
        wv_sb = [const.tile([128, HD], bf16, name=f"wvsb{c}") for c in range(DCH)]
        wo_sb = [const.tile([128, DIM], bf16, name=f"wosb{c}") for c in range(4)]
        # Q^T / K^T packed: chunk c holds heads 2c (parts 0-63) and 2c+1 (64-127)
        QT = [const.tile([128, N], bf16, name=f"QTsb{c}") for c in range(4)]
        KT = [const.tile([128, N], bf16, name=f"KTsb{c}") for c in range(4)]
        # V_aug: per k-tile block of 8*(64+1) cols
        V = const.tile([128, NKT * VROW], bf16, name="Vsb")
        # O^T packed like QT/KT
        OT = [const.tile([128, N], bf16, name=f"OTsb{c}") for c in range(4)]
        cmask = const.tile([128, 4 * QCHUNK], bf16, name="cmasksb")
        # key-padding mask: col t = mask[t*128 + p] (one tiny DMA, loaded
        # first so V-proj evacuations never wait behind the big weight loads)
        kpm_sb = const.tile([128, NKT], f32, name="kpmsb")

        sync = nc.sync
        sync.dma_start(
            kpm_sb.rearrange("p (t one) -> p t one", one=1),
            kpm.rearrange("(t p) one -> p t one", p=128),
        )

        # ---- load inputs (wv+xT first so V projection can start ASAP;
        # full-chunk DMAs only — small column-split loads are inefficient) ----
        for c in range(DCH):
            sync.dma_start(wv_sb[c][:], wv[c * 128:(c + 1) * 128, :])
            sync.dma_start(xT_sb[c][:], xT[c * 128:(c + 1) * 128, :])
        for c in range(DCH):
            sync.dma_start(wq_sb[c][:], wq[c * 128:(c + 1) * 128, :])
            sync.dma_start(wk_sb[c][:], wk[c * 128:(c + 1) * 128, :])
        for c in range(4):
            sync.dma_start(wo_sb[c][:], wo[c * 128:(c + 1) * 128, :])
        # cmask DRAM row r*128+k, col q  ->  SBUF part k, col r*512+q
        sync.dma_start(
            cmask.rearrange("p (r q) -> p r q", r=4),
            cmask_d.rearrange("(r p) q -> p r q", p=128),
        )

        # ---- V projection: V[tok, dh] via x-stationary matmuls ----
        def v_proj():
            for kt in range(NKT):
                kpm_t = kpm_sb[:, kt:kt + 1]
                ps = mm_ps.tile([128, 512], f32, tag="mm", name="ps")
                for c in range(DCH):
                    nc.tensor.matmul(
                        ps[:], xT_sb[c][:, kt * 128:(kt + 1) * 128],
                        wv_sb[c][:],
                        start=(c == 0), stop=(c == DCH - 1))
                vblk = V[:, kt * VROW:(kt + 1) * VROW].rearrange(
                    "p (h c) -> p h c", c=DH + 1)
                # data cols, scaled by key-padding mask
                nc.vector.tensor_scalar_mul(
                    vblk[:, :, 0:DH],
                    ps.rearrange("p (h c) -> p h c", c=DH),
                    kpm_t[:, 0:1])
                # ones column = mask value (free-dim stride-0 broadcast read)
                nc.vector.tensor_copy(vblk[:, :, DH:DH + 1].squeeze(),
                                      kpm_t[:, 0:1].broadcast_to([128, HPC]))

        def qk_proj(c):
            for tcx in range(NQC):
                tsl = slice(tcx * QCHUNK, (tcx + 1) * QCHUNK)
                psq = mm_ps.tile([128, 512], f32, tag="mm", name="psq")
                for d in range(DCH):
                    nc.tensor.matmul(
                        psq[:], wq_sb[d][:, c * 128:(c + 1) * 128],
                        xT_sb[d][:, tsl],
                        start=(d == 0), stop=(d == DCH - 1))
                nc.vector.tensor_copy(QT[c][:, tsl], psq[:])
                psk = mm_ps.tile([128, 512], f32, tag="mm", name="psk")
                for d in range(DCH):
                    nc.tensor.matmul(
                        psk[:], wk_sb[d][:, c * 128:(c + 1) * 128],
                        xT_sb[d][:, tsl],
                        start=(d == 0), stop=(d == DCH - 1))
                nc.vector.tensor_copy(KT[c][:, tsl], psk[:])

        def attend(h, qc):
            c = h // 2
            po = (h % 2) * 64          # partition offset within chunk
            qt_h = QT[c][po:po + 64, :]
            kt_h = KT[c][po:po + 64, :]
            qsl = slice(qc * QCHUNK, (qc + 1) * QCHUNK)
            pso = o_ps.tile([DH + 1, 512], f32, tag="o", name="pso")
            nkt = 4 * qc + 4
            for kp in range(nkt // 2):
                ps2 = s_ps.tile([128, 1024], f32, tag="s", name="ps2")
                for j in (0, 1):
                    kt = 2 * kp + j
                    nc.tensor.matmul(
                        ps2[:, j * 512:(j + 1) * 512],
                        kt_h[:, kt * 128:(kt + 1) * 128],
                        qt_h[:, qsl],
                        start=True, stop=True)
                p2 = p_sbp.tile([128, 1024], bf16, tag="p", name="p2")
                nc.scalar.activation(p2[:], ps2[:], Exp)
                r = 2 * kp - 4 * qc
                if r >= 0:  # diagonal pair: apply causal 0/1 mask
                    nc.vector.tensor_mul(
                        p2[:], p2[:],
                        cmask[:, r * QCHUNK:(r + 2) * QCHUNK])
                for j in (0, 1):
                    kt = 2 * kp + j
                    nc.tensor.matmul(
                        pso[:],
                        V[:, kt * VROW + h * (DH + 1):
                           kt * VROW + (h + 1) * (DH + 1)],
                        p2[:, j * 512:(j + 1) * 512],
                        start=(kt == 0), stop=(kt == nkt - 1),
                        skip_group_check=True)
            # normalize: O^T[0:64] * (1 / rowsum row 64)
            # (stage rowsum into SBUF: custom-DVE recip needs SBUF in)
            rsum = miscp.tile([1, 512], f32, tag="rsum", name="rsum")
            nc.vector.tensor_copy(rsum[:], pso[DH:DH + 1, :])
            recip = miscp.tile([1, 512], f32, tag="recip", name="recip")
            nc.vector.reciprocal_approx_fast(recip[:], rsum[:])
            bcast = miscp.tile([64, 512], f32, tag="bcast", name="bcast")
            nc.gpsimd.partition_broadcast(bcast[:], recip[:])
            if po == 0:
                nc.vector.tensor_mul(OT[c][0:64, qsl],
                                     pso[0:DH, :], bcast[:])
            else:
                otmp = miscp.tile([64, 512], bf16, tag="otmp", bufs=3,
                                  name="otmp")
                nc.vector.tensor_mul(otmp[:], pso[0:DH, :], bcast[:])
                # partition shift 0->64 needs a DMA, engines can't shift
                sync.dma_start(OT[c][64:128, qsl], otmp[:])

        def out_proj(qt):
            y_sb = outp.tile([128, DIM], f32, tag="y", name="y_sb")
            for oc in range(2):
                psy = mm_ps.tile([128, 512], f32, tag="mm", name="psy")
                for cc in range(4):
                    nc.tensor.matmul(
                        psy[:], OT[cc][:, qt * 128:(qt + 1) * 128],
                        wo_sb[cc][:, oc * 512:(oc + 1) * 512],
                        start=(cc == 0), stop=(cc == 3))
                nc.vector.tensor_copy(y_sb[:, oc * 512:(oc + 1) * 512],
                                      psy[:])
            sync.dma_start(out_d[qt * 128:(qt + 1) * 128, :], y_sb[:])

        # ---- proj chunk c then its two heads (overlaps ACT exp with PE
        # proj of later chunks); last pair goes qc-major with inline Y ----
        v_proj()
        qk_proj(0)
        for h in (0, 1):
            for qc in range(NQC):
                attend(h, qc)
        for c in (1, 2):
            qk_proj(c)
            for h in (2 * c, 2 * c + 1):
                for qc in range(NQC):
                    attend(h, qc)
        qk_proj(3)
        for qc in range(NQC):
            attend(7, qc)
            attend(6, qc)
            for qt in range(4 * qc, 4 * qc + 4):
                out_proj(qt)

    nc.compile()
    return nc


def _get_program():
    if "nc" not in _CACHE:
        _CACHE["nc"] = _build_program()
    return _CACHE["nc"]


def _prep_inputs(x, mask, w_qkv, w_out):
    """Build the 8 per-core input maps (host-side sharding)."""
    scale = DH ** -0.5
    # causal keep-mask patterns for the 4 diagonal k-tiles of a 512 q-chunk
    k_idx = np.arange(128)[:, None]
    q_idx = np.arange(QCHUNK)[None, :]
    cm = np.concatenate(
        [(q_idx >= r * 128 + k_idx) for r in range(4)], axis=0
    ).astype(BF16)  # [512, 512]

    xT = [np.ascontiguousarray(x[b].T).astype(BF16) for b in range(B)]
    in_maps = []
    for core in range(NCORES):
        b, hg = core // 2, core % 2
        cs = slice(hg * HD, (hg + 1) * HD)
        wq_s = (w_qkv[:, 0 * DIM:1 * DIM][:, cs] * scale).astype(BF16)
        wk_s = w_qkv[:, 1 * DIM:2 * DIM][:, cs].astype(BF16)
        wv_s = w_qkv[:, 2 * DIM:3 * DIM][:, cs].astype(BF16)
        wo_s = np.ascontiguousarray(w_out[cs, :]).astype(BF16)
        kpm = mask[b].astype(np.float32).reshape(N, 1)
        in_maps.append({
            "xT": xT[b], "wq": wq_s, "wk": wk_s, "wv": wv_s, "wo": wo_s,
            "kpm": np.ascontiguousarray(kpm), "cmask": cm,
        })
    return in_maps


def kernel(x, mask, w_qkv, w_out, b_out, _trace=False):
    from concourse import bass_utils

    x = np.asarray(x, dtype=np.float32)
    mask = np.asarray(mask)
    w_qkv = np.asarray(w_qkv, dtype=np.float32)
    w_out = np.asarray(w_out, dtype=np.float32)
    b_out = np.asarray(b_out, dtype=np.float32)

    nc = _get_program()
    in_maps = _prep_inputs(x, mask, w_qkv, w_out)
    res = bass_utils.run_bass_kernel_spmd(
        nc, in_maps, core_ids=list(range(NCORES)), trace=_trace)

    out = np.empty((B, N, DIM), dtype=np.float32)
    for b in range(B):
        out[b] = res.results[2 * b]["out"] + res.results[2 * b + 1]["out"] + b_out
    if _trace:
        return out, res
    return out
